# revision 6
# baseline (speedup 1.0000x reference)
"""Trainium2 Bass kernel v2 for HME-VideoQA multi-modal attention GRU.

Design vs baseline:
- ONE AllReduce per iteration: payload [cv | ct | gh-partials | hWhh-partials
  | hWb-partials | Z]. Small weights (Uav/Uat/W_ihT/Wvh/Wth) replicated per
  core so hu/gi/u GEMVs are local; h kept fully replicated on every core.
- All GEMVs row-form (vector stationary lhsT [128,1], weight streams as rhs)
  with 4-way PE column tiling (tile_position) -> 4 concurrent 512-col chains.
- bf16 streams (U matrices fp8), fp32 PSUM accumulation, fp32 gate math.
- Layout changes via fold-DMAs (contiguous segments) + PE transposes.
"""

import numpy as np
import ml_dtypes
from contextlib import ExitStack

H = 1024
P = 128
NCORES = 8
KB = H // P             # 8 H-blocks
TVC = 8192 // NCORES    # 1024 video slots/core
TTC = 2048 // NCORES    # 256 text slots/core
SVB = TVC // P          # 8 video slot blocks
STB = TTC // P          # 2 text slot blocks

# AR payload offsets (fp32). it>0: [cv|ct|gh|hwhh|wb|z]; it0: [cv|ct|z]
AB_CV, AB_CT, AB_GH, AB_HW, AB_WB, AB_Z = 0, 1024, 2048, 5120, 6144, 6146
AB_LEN = 6149
A0_Z = 2048
A0_LEN = 2051

_cache = {}


def _build(loop_n):
    import concourse.bacc as bacc
    import concourse.mybir as mybir
    import concourse.tile as tile
    import concourse.bass as bass  # noqa: F401

    nc = bacc.Bacc("TRN2", target_bir_lowering=False, debug=False,
                   num_devices=NCORES)
    f32 = mybir.dt.float32
    bf16 = mybir.dt.bfloat16
    fp8 = mybir.dt.float8e4
    AF = mybir.ActivationFunctionType
    ALU = mybir.AluOpType
    RG = [list(range(NCORES))]

    def din(name, shape, dty):
        return nc.dram_tensor(name, list(shape), dty,
                              kind="ExternalInput").ap()

    memTv_in = din("memTv", [P, KB * TVC], bf16)
    memTt_in = din("memTt", [P, KB * TTC], bf16)
    memRv_in = din("memRv", [P, SVB * H], bf16)
    memRt_in = din("memRt", [P, STB * H], bf16)
    wavT_in = din("wavT", [P, KB * H], bf16)
    watT_in = din("watT", [P, KB * H], bf16)
    uavR_in = din("uavR", [P, KB * H], fp8)
    uatR_in = din("uatR", [P, KB * H], fp8)
    wvhR_in = din("wvhR", [P, KB * H], bf16)
    wthR_in = din("wthR", [P, KB * H], bf16)
    wihTR_in = din("wihTR", [P, KB * 3 * H], bf16)
    # sharded rhs for h-partials: [W_hh.T rows | Whh rows | Wb rows]
    whhTs_in = din("whhTs", [P, 4 * H + 2], bf16)
    vavB_in = din("vavB", [P, KB], bf16)
    vatB_in = din("vatB", [P, KB], bf16)
    bavB_in = din("bavB", [P, KB], f32)
    batB_in = din("batB", [P, KB], f32)
    bhhF_in = din("bhhF", [KB, P], f32)
    gbF_in = din("gbF", [P, 4 * KB], f32)
    bbS_in = din("bbS", [1, 2], f32)
    maskB_in = din("maskB", [P, KB], bf16)
    ones18_in = din("ones18", [1, KB], f32)
    eyeB_in = din("eyeB", [32, 32], bf16)
    eyeF_in = din("eyeF", [32, 32], f32)
    h_out = nc.dram_tensor("h_out", [P, KB], f32, kind="ExternalOutput").ap()

    with tile.TileContext(nc) as tc, ExitStack() as ctx:
        cst = ctx.enter_context(tc.tile_pool(name="cst", bufs=1))
        wgt = ctx.enter_context(tc.tile_pool(name="wgt", bufs=1))
        res = ctx.enter_context(tc.tile_pool(name="res", bufs=1))
        dram = ctx.enter_context(tc.tile_pool(name="dram", bufs=2,
                                              space="DRAM"))
        pbig = ctx.enter_context(tc.tile_pool(name="pbig", bufs=5,
                                              space="PSUM"))
        psm = ctx.enter_context(tc.tile_pool(name="psm", bufs=2,
                                             space="PSUM"))
        thp = ctx.enter_context(tc.tile_pool(name="thp", bufs=2))
        wk = ctx.enter_context(tc.tile_pool(name="wk", bufs=1))
        hhp = ctx.enter_context(tc.tile_pool(name="hh", bufs=2))
        stg = ctx.enter_context(tc.tile_pool(name="stg", bufs=3))

        # memT first on the sync ring: it gates the setup GEMM. The small
        # constants ride behind it (not needed until iter0's tanh).
        memT = wgt.tile([P, KB * TVC], bf16, tag="memT", name="memT")
        nc.sync.dma_start(memT[:], memTv_in)

        # ---- constants (scalar ring; sync ring carries the big loads) ----
        def cload(pool, ap_in, shape, dty, tag):
            t = pool.tile(list(shape), dty, tag=tag, name=tag)
            nc.scalar.dma_start(t[:], ap_in)
            return t

        vavB = cload(cst, vavB_in, [P, KB], bf16, "vavB")
        vatB = cload(cst, vatB_in, [P, KB], bf16, "vatB")
        bavB = cload(cst, bavB_in, [P, KB], f32, "bavB")
        batB = cload(cst, batB_in, [P, KB], f32, "batB")
        bhhF = cload(cst, bhhF_in, [KB, P], f32, "bhhF")
        gbF = cload(cst, gbF_in, [P, 4 * KB], f32, "gbF")
        bbS = cload(cst, bbS_in, [1, 2], f32, "bbS")
        maskB = cload(cst, maskB_in, [P, KB], bf16, "maskB")
        ones18 = cload(cst, ones18_in, [1, KB], f32, "ones18")
        eyeB = cload(cst, eyeB_in, [32, 32], bf16, "eyeB")
        eyeF = cload(cst, eyeF_in, [32, 32], f32, "eyeF")

        zeros8F = cst.tile([KB, P], f32, tag="zeros8F", name="zeros8F")
        nc.vector.memset(zeros8F[:], 0.0)
        wrm = cst.tile([P, P], bf16, tag="wrm", name="wrm")
        nc.vector.memset(wrm[:], 0.0)
        zeros12 = cst.tile([1, 2], f32, tag="zeros12", name="zeros12")
        nc.vector.memset(zeros12[:], 0.0)
        zeros24 = cst.tile([P, 3 * KB], f32, tag="zeros24", name="zeros24")
        nc.vector.memset(zeros24[:], 0.0)

        # ---- big DMAs (sync ring, in need order) ----
        memTtS = wgt.tile([P, KB * TTC], bf16, tag="memTt2", name="memTtS")
        nc.sync.dma_start(memTtS[:], memTt_in)

        mvw = res.tile([P, KB * TVC], bf16, tag="mvw", name="mvw")
        mtw = res.tile([P, KB * TTC], bf16, tag="mtw", name="mtw")

        def mm(out, lhsT, rhs, tp, start, stop):
            nc.tensor.matmul(out, lhsT, rhs, start=start, stop=stop,
                             tile_position=tp, skip_group_check=True)

        def flush(bank):
            """DVE-copy a PSUM bank to SBUF (DMA cannot read PSUM)."""
            s = stg.tile([P, 512], f32, tag="stg", name="s")
            nc.vector.tensor_copy(s[:], bank[:])
            return s

        # ---- setup GEMM (video): mvw[jb-block] = (mem@Wav).T layout ----
        # w half jh: blocks kb hold Wav[kb-rows, jh*512:(jh+1)*512]
        for jh in range(2):
            wv = wgt.tile([P, KB * 512], bf16, tag="wtag", name="wv",
                          bufs=2)
            nc.sync.dma_start(wv[:], wavT_in[:, jh * 4096:(jh + 1) * 4096])
            for jj in range(4):
                jb = jh * 4 + jj
                for pc in range(2):
                    ps = pbig.tile([P, 512], f32, tag="big", name="ps")
                    for kb in range(KB):
                        nc.tensor.matmul(
                            ps[:],
                            wv[:, kb * 512 + jj * P: kb * 512 + (jj + 1) * P],
                            memT[:, kb * TVC + pc * 512:
                                 kb * TVC + (pc + 1) * 512],
                            start=(kb == 0), stop=(kb == KB - 1))
                    nc.vector.tensor_copy(
                        mvw[:, jb * TVC + pc * 512: jb * TVC + (pc + 1) * 512],
                        ps[:])

        # text GEMM
        for jh in range(2):
            wt = wgt.tile([P, KB * 512], bf16, tag="wtag", name="wt",
                          bufs=2)
            nc.sync.dma_start(wt[:], watT_in[:, jh * 4096:(jh + 1) * 4096])
            for jj in range(4):
                jb = jh * 4 + jj
                ps = pbig.tile([P, TTC], f32, tag="big", name="ps")
                for kb in range(KB):
                    nc.tensor.matmul(
                        ps[:],
                        wt[:, kb * 512 + jj * P: kb * 512 + (jj + 1) * P],
                        memTtS[:, kb * TTC: (kb + 1) * TTC],
                        start=(kb == 0), stop=(kb == KB - 1))
                nc.vector.tensor_copy(mtw[:, jb * TTC:(jb + 1) * TTC], ps[:])

        # remaining weights (needed from iter0 context onward)
        memRv = wgt.tile([P, SVB * H], bf16, tag="memRv", name="memRv")
        nc.sync.dma_start(memRv[:], memRv_in)
        memRt = wgt.tile([P, STB * H], bf16, tag="memRt", name="memRt")
        nc.sync.dma_start(memRt[:], memRt_in)
        wvhR = wgt.tile([P, KB * H], bf16, tag="wvhR", name="wvhR")
        nc.sync.dma_start(wvhR[:], wvhR_in)
        wthR = wgt.tile([P, KB * H], bf16, tag="wthR", name="wthR")
        nc.sync.dma_start(wthR[:], wthR_in)
        wihTR = wgt.tile([P, KB * 3 * H], bf16, tag="wihTR", name="wihTR")
        nc.sync.dma_start(wihTR[:], wihTR_in)
        uavR = wgt.tile([P, KB * H], fp8, tag="uavR", name="uavR")
        nc.sync.dma_start(uavR[:], uavR_in)
        uatR = wgt.tile([P, KB * H], fp8, tag="uatR", name="uatR")
        nc.sync.dma_start(uatR[:], uatR_in)
        whhTs = wgt.tile([P, 4 * H + 2], bf16, tag="whhTs", name="whhTs")
        nc.sync.dma_start(whhTs[:], whhTs_in)

        # ---- recurrence ----
        hC = None     # [P, KB] f32, full h, col-block layout
        hB = None     # bf16 copy
        hB8 = None    # fp8 copy (lhsT for fp8 U GEMVs)

        for it in range(loop_n):
            first = (it == 0)

            if not first:
                # --- phase 1a: local hu GEMVs (fp8 U, replicated) ---
                g1 = pbig.tile([P, 512], f32, tag="big", name="g1")
                rhs_map = [(uavR, 0), (uavR, 512), (uatR, 0), (uatR, 512)]
                for kb in range(KB):
                    hcol = hB8[:, kb:kb + 1]
                    for j, (w, off) in enumerate(rhs_map):
                        mm(g1[32 * j:32 * j + 1, :], hcol,
                           w[:, kb * H + off: kb * H + off + 512],
                           (0, 32 * j), kb == 0, kb == KB - 1)

                # --- phase 1b: sharded partials (gh, hWhh, hWb) ---
                hsel = wk.tile([P, 1], f32, tag="hsel", name="hsel")
                msk = wk.tile([P, KB], f32, tag="msk", name="msk")
                nc.vector.tensor_tensor(msk[:], hB[:], maskB[:], op=ALU.mult)
                nc.vector.tensor_reduce(hsel[:], msk[:],
                                        axis=mybir.AxisListType.XYZW,
                                        op=ALU.add)
                hselB = wk.tile([P, 1], bf16, tag="hselB", name="hselB")
                nc.vector.tensor_copy(hselB[:], hsel[:])

                g3 = pbig.tile([P, 512], f32, tag="big", name="g3")
                g4 = pbig.tile([P, 512], f32, tag="big", name="g4")
                for j in range(4):
                    mm(g3[32 * j:32 * j + 1, :], hselB[:],
                       whhTs[:, j * 512:(j + 1) * 512],
                       (0, 32 * j), True, True)
                for j in range(4):
                    mm(g4[32 * j:32 * j + 1, :], hselB[:],
                       whhTs[:, (4 + j) * 512:(5 + j) * 512],
                       (0, 32 * j), True, True)
                pwb = psm.tile([P, 32], f32, tag="smF", name="pwb")
                mm(pwb[0:1, 0:2], hselB[:], whhTs[:, 4 * H:4 * H + 2],
                   (0, 0), True, True)

                # fold hu FIRST (gates the tanh phase; scalar ring)
                sg1 = flush(g1)
                huF = wk.tile([2 * KB, P], f32, tag="huF", name="huF")
                nc.scalar.dma_start(huF[:], sg1[0:128:32, :])

                # stage partials into AR input (gpsimd ring, needed later)
                sg3 = flush(g3)
                sg4 = flush(g4)
                spwb = wk.tile([1, 2], f32, tag="spwb", name="spwb")
                nc.vector.tensor_copy(spwb[:], pwb[0:1, 0:2])
                arin = dram.tile([1, AB_LEN], f32, tag="arin", name="arin")
                nc.scalar.dma_start(arin[0, AB_GH:AB_GH + 2048],
                                    sg3[0:128:32, :])
                nc.scalar.dma_start(arin[0, AB_GH + 2048:AB_GH + 4096],
                                    sg4[0:128:32, :])
                nc.scalar.dma_start(arin[0, AB_WB:AB_WB + 2], spwb[:])

                pt = psm.tile([P, 32], f32, tag="smF", name="pt")
                nc.tensor.transpose(pt[:, 0:2 * KB], huF[:],
                                    eyeF[0:2 * KB, 0:2 * KB])
                biasV = wk.tile([P, KB], f32, tag="biasV", name="biasV")
                biasT = wk.tile([P, KB], f32, tag="biasT", name="biasT")
                nc.vector.tensor_tensor(biasV[:], pt[:, 0:KB], bavB[:],
                                        op=ALU.add)
                nc.vector.tensor_tensor(biasT[:], pt[:, KB:2 * KB], batB[:],
                                        op=ALU.add)
            else:
                biasV, biasT = bavB, batB
                arin = dram.tile([1, A0_LEN], f32, tag="arin0", name="arin0")

            # --- tanh + scores ---
            sc = pbig.tile([P, 512], f32, tag="big", name="sc")
            for kb in range(KB):
                th = thp.tile([P, TVC], bf16, tag="thv", name="th")
                nc.scalar.activation(th[:], mvw[:, kb * TVC:(kb + 1) * TVC],
                                     AF.Tanh, bias=biasV[:, kb:kb + 1])
                mm(sc[0:1, :], vavB[:, kb:kb + 1], th[:, 0:512],
                   (0, 0), kb == 0, kb == KB - 1)
                mm(sc[32:33, :], vavB[:, kb:kb + 1], th[:, 512:1024],
                   (0, 32), kb == 0, kb == KB - 1)
            for kb in range(KB):
                tht = thp.tile([P, TTC], bf16, tag="tht", name="tht")
                nc.scalar.activation(tht[:], mtw[:, kb * TTC:(kb + 1) * TTC],
                                     AF.Tanh, bias=biasT[:, kb:kb + 1])
                mm(sc[64:65, 0:TTC], vatB[:, kb:kb + 1], tht[:],
                   (0, 64), kb == 0, kb == KB - 1)

            # --- exp + Z ---
            evS = wk.tile([33, 512], bf16, tag="evS", name="evS")
            accS = wk.tile([33, 1], f32, tag="accS", name="accS")
            nc.scalar.activation(evS[:], sc[0:33, :], AF.Exp,
                                 accum_out=accS[:])
            etS = wk.tile([1, TTC], bf16, tag="etS", name="etS")
            accT = wk.tile([1, 1], f32, tag="accT", name="accT")
            nc.scalar.activation(etS[:], sc[64:65, 0:TTC], AF.Exp,
                                 accum_out=accT[:])
            zoff = A0_Z if first else AB_Z
            nc.scalar.dma_start(arin[0, zoff:zoff + 1], accS[0:1, :])
            nc.scalar.dma_start(arin[0, zoff + 1:zoff + 2], accS[32:33, :])
            nc.scalar.dma_start(arin[0, zoff + 2:zoff + 3], accT[0:1, :])

            # fold ev -> [10,128], transpose -> evT [128,10]
            evF = wk.tile([SVB + STB, P], bf16, tag="evF", name="evF")
            nc.scalar.dma_start(evF[0:4, :], evS[0:1, :])
            nc.scalar.dma_start(evF[4:8, :], evS[32:33, :])
            nc.scalar.dma_start(evF[8:10, :], etS[:])
            ptev = psm.tile([P, 32], bf16, tag="smB", name="ptev", bufs=1)
            nc.tensor.transpose(ptev[:, 0:SVB + STB], evF[:],
                                eyeB[0:SVB + STB, 0:SVB + STB])
            evT = wk.tile([P, SVB + STB], bf16, tag="evT", name="evT")
            nc.vector.tensor_copy(evT[:], ptev[:, 0:SVB + STB])

            # --- context ---
            cx = pbig.tile([P, 512], f32, tag="big", name="cx")
            for sb in range(SVB):
                mm(cx[0:1, :], evT[:, sb:sb + 1],
                   memRv[:, sb * H: sb * H + 512],
                   (0, 0), sb == 0, sb == SVB - 1)
                mm(cx[32:33, :], evT[:, sb:sb + 1],
                   memRv[:, sb * H + 512: (sb + 1) * H],
                   (0, 32), sb == 0, sb == SVB - 1)
            for sb in range(STB):
                mm(cx[64:65, :], evT[:, SVB + sb:SVB + sb + 1],
                   memRt[:, sb * H: sb * H + 512],
                   (0, 64), sb == 0, sb == STB - 1)
                mm(cx[96:97, :], evT[:, SVB + sb:SVB + sb + 1],
                   memRt[:, sb * H + 512: (sb + 1) * H],
                   (0, 96), sb == 0, sb == STB - 1)
            scx = flush(cx)
            nc.scalar.dma_start(arin[0, 0:2048], scx[0:128:32, :])

            # --- AllReduce ---
            if first:
                arout = dram.tile([1, A0_LEN], f32, tag="arout0",
                                  name="arout0", addr_space="Shared")
            else:
                arout = dram.tile([1, AB_LEN], f32, tag="arout",
                                  name="arout", addr_space="Shared")
            nc.gpsimd.collective_compute(
                "AllReduce", ALU.add, replica_groups=RG,
                ins=[arin.opt()], outs=[arout.opt()])

            # --- unstage ---
            cvctF = wk.tile([2 * KB, P], f32, tag="cvctF", name="cvctF")
            nc.scalar.dma_start(cvctF[:], arout[0, AB_CV:AB_CV + 2 * H])
            zS = wk.tile([1, 3], f32, tag="zS", name="zS")
            nc.scalar.dma_start(zS[:], arout[0, zoff:zoff + 3])
            if not first:
                ghF = wk.tile([3 * KB, P], f32, tag="ghF", name="ghF")
                nc.sync.dma_start(ghF[:], arout[0, AB_GH:AB_GH + 3 * H])
                hWhhF = wk.tile([KB, P], f32, tag="hWhhF", name="hWhhF")
                nc.sync.dma_start(hWhhF[:], arout[0, AB_HW:AB_HW + H])
                hwbS = wk.tile([1, 2], f32, tag="hwbS", name="hwbS")
                nc.scalar.dma_start(hwbS[:], arout[0, AB_WB:AB_WB + 2])
            else:
                hwbS = zeros12
                hWhhF = zeros8F

            # --- beta / Z scalars ---
            bsum = wk.tile([1, 2], f32, tag="bsum", name="bsum")
            nc.vector.tensor_tensor(bsum[:], hwbS[:], bbS[:], op=ALU.add)
            eb = wk.tile([1, 2], f32, tag="eb", name="eb")
            ebs = wk.tile([1, 1], f32, tag="ebs", name="ebs")
            nc.scalar.activation(eb[:], bsum[:], AF.Exp, accum_out=ebs[:])
            erec = wk.tile([1, 1], f32, tag="erec", name="erec")
            nc.vector.reciprocal(erec[:], ebs[:])
            beta = wk.tile([1, 2], f32, tag="beta", name="beta")
            nc.vector.tensor_scalar_mul(beta[:], eb[:], erec[:])
            zz = wk.tile([1, 2], f32, tag="zz", name="zz")
            nc.vector.tensor_tensor(zz[:, 0:1], zS[:, 0:1], zS[:, 1:2],
                                    op=ALU.add)
            nc.vector.tensor_copy(zz[:, 1:2], zS[:, 2:3])
            zrec = wk.tile([1, 2], f32, tag="zrec", name="zrec")
            nc.vector.reciprocal(zrec[:], zz[:])
            rr = wk.tile([1, 2], f32, tag="rr", name="rr")
            nc.vector.tensor_tensor(rr[:], beta[:], zrec[:], op=ALU.mult)
            prr = psm.tile([P, 32], f32, tag="smF", name="prr")
            nc.tensor.matmul(prr[0:KB, 0:2], ones18[:], rr[:],
                             start=True, stop=True)
            rrB = wk.tile([KB, 2], f32, tag="rrB", name="rrB")
            nc.vector.tensor_copy(rrB[:], prr[0:KB, 0:2])

            # --- u-GEMV: u = cv @ Wvh, ut = ct @ Wth (replicated) ---
            ptcv = psm.tile([P, 32], f32, tag="smF", name="ptcv")
            nc.tensor.transpose(ptcv[:, 0:2 * KB], cvctF[:],
                                eyeF[0:2 * KB, 0:2 * KB])
            cvctB = wk.tile([P, 2 * KB], bf16, tag="cvctB", name="cvctB")
            nc.vector.tensor_copy(cvctB[:], ptcv[:, 0:2 * KB])
            ub = pbig.tile([P, 512], f32, tag="big", name="ub")
            for kb in range(KB):
                mm(ub[0:1, :], cvctB[:, kb:kb + 1],
                   wvhR[:, kb * H: kb * H + 512],
                   (0, 0), kb == 0, kb == KB - 1)
                mm(ub[32:33, :], cvctB[:, kb:kb + 1],
                   wvhR[:, kb * H + 512: (kb + 1) * H],
                   (0, 32), kb == 0, kb == KB - 1)
                mm(ub[64:65, :], cvctB[:, KB + kb:KB + kb + 1],
                   wthR[:, kb * H: kb * H + 512],
                   (0, 64), kb == 0, kb == KB - 1)
                mm(ub[96:97, :], cvctB[:, KB + kb:KB + kb + 1],
                   wthR[:, kb * H + 512: (kb + 1) * H],
                   (0, 96), kb == 0, kb == KB - 1)

            # --- mm_o (folded rows [8,128]) ---
            sub = flush(ub)
            uFv = wk.tile([KB, P], f32, tag="uFv", name="uFv")
            uFt = wk.tile([KB, P], f32, tag="uFt", name="uFt")
            nc.scalar.dma_start(uFv[:], sub[0:64:32, :])
            nc.scalar.dma_start(uFt[:], sub[64:128:32, :])
            hwb_b = wk.tile([KB, P], f32, tag="hwb_b", name="hwb_b")
            nc.vector.tensor_tensor(hwb_b[:], hWhhF[:], bhhF[:], op=ALU.add)
            t1 = wk.tile([KB, P], f32, tag="t1", name="t1")
            nc.vector.scalar_tensor_tensor(t1[:], uFv[:], rrB[:, 0:1],
                                           hwb_b[:], op0=ALU.mult,
                                           op1=ALU.add)
            t2 = wk.tile([KB, P], f32, tag="t2", name="t2")
            nc.vector.scalar_tensor_tensor(t2[:], uFt[:],
                                           rrB[:, 1:2], t1[:],
                                           op0=ALU.mult, op1=ALU.add)
            moF = wk.tile([KB, P], bf16, tag="moF", name="moF")
            nc.scalar.activation(moF[:], t2[:], AF.Tanh)
            ptmo = psm.tile([P, 32], bf16, tag="smB", name="ptmo", bufs=1)
            nc.tensor.transpose(ptmo[:, 0:KB], moF[:], eyeB[0:KB, 0:KB])
            moB = wk.tile([P, KB], bf16, tag="moB", name="moB")
            nc.vector.tensor_copy(moB[:], ptmo[:, 0:KB])

            # --- gi GEMV: gi = mo @ W_ih.T (full 3H, local) ---
            giE = pbig.tile([P, 512], f32, tag="big", name="giE")
            giF_ = pbig.tile([P, 512], f32, tag="big", name="giF_")
            for kb in range(KB):
                mo_col = moB[:, kb:kb + 1]
                for pc in range(4):
                    mm(giE[32 * pc:32 * pc + 1, :], mo_col,
                       wihTR[:, kb * 3 * H + pc * 512:
                             kb * 3 * H + (pc + 1) * 512],
                       (0, 32 * pc), kb == 0, kb == KB - 1)
                for pc in range(2):
                    mm(giF_[32 * pc:32 * pc + 1, :], mo_col,
                       wihTR[:, kb * 3 * H + (4 + pc) * 512:
                             kb * 3 * H + (5 + pc) * 512],
                       (0, 32 * pc), kb == 0, kb == KB - 1)

            # fold gi -> [24,128] -> transpose -> giC [128,24]
            sgiE = flush(giE)
            sgiF = flush(giF_)
            giFld = wk.tile([3 * KB, P], f32, tag="giFld", name="giFld")
            nc.scalar.dma_start(giFld[0:16, :], sgiE[0:128:32, :])
            nc.scalar.dma_start(giFld[16:24, :], sgiF[0:64:32, :])
            ptgi = psm.tile([P, 32], f32, tag="smF", name="ptgi")
            nc.tensor.transpose(ptgi[:, 0:3 * KB], giFld[:],
                                eyeF[0:3 * KB, 0:3 * KB])
            giC = wk.tile([P, 3 * KB], f32, tag="giC", name="giC")
            nc.vector.tensor_copy(giC[:], ptgi[:, 0:3 * KB])

            if not first:
                ptgh = psm.tile([P, 32], f32, tag="smF", name="ptgh")
                nc.tensor.transpose(ptgh[:, 0:3 * KB], ghF[:],
                                    eyeF[0:3 * KB, 0:3 * KB])
                ghC = wk.tile([P, 3 * KB], f32, tag="ghC", name="ghC")
                nc.vector.tensor_copy(ghC[:], ptgh[:, 0:3 * KB])
            else:
                ghC = zeros24

            # --- gates (columns; r cols 0-7, z 8-15, n 16-23) ---
            pre = wk.tile([P, 2 * KB], f32, tag="pre", name="pre")
            nc.vector.tensor_tensor(pre[:], giC[:, 0:2 * KB],
                                    ghC[:, 0:2 * KB], op=ALU.add)
            nc.vector.tensor_tensor(pre[:], pre[:], gbF[:, 0:2 * KB],
                                    op=ALU.add)
            erz = wk.tile([P, 2 * KB], f32, tag="erz", name="erz")
            nc.scalar.activation(erz[:], pre[:], AF.Exp, scale=-1.0)
            nc.vector.tensor_scalar_add(erz[:], erz[:], 1.0)
            rz = wk.tile([P, 2 * KB], f32, tag="rz", name="rz")
            nc.vector.reciprocal(rz[:], erz[:])
            hn = wk.tile([P, KB], f32, tag="hn", name="hn")
            nc.vector.tensor_tensor(hn[:], ghC[:, 2 * KB:3 * KB],
                                    gbF[:, 3 * KB:4 * KB], op=ALU.add)
            m1 = wk.tile([P, KB], f32, tag="m1", name="m1")
            nc.vector.tensor_tensor(m1[:], rz[:, 0:KB], hn[:], op=ALU.mult)
            tn = wk.tile([P, KB], f32, tag="tn", name="tn")
            nc.vector.tensor_tensor(tn[:], giC[:, 2 * KB:3 * KB],
                                    gbF[:, 2 * KB:3 * KB], op=ALU.add)
            nc.vector.tensor_tensor(tn[:], tn[:], m1[:], op=ALU.add)
            ng = wk.tile([P, KB], f32, tag="ng", name="ng")
            nc.scalar.activation(ng[:], tn[:], AF.Tanh)
            hC_new = hhp.tile([P, KB], f32, tag="hC", name="hC")
            d = wk.tile([P, KB], f32, tag="d", name="d")
            if first:
                nc.vector.tensor_tensor(d[:], rz[:, KB:2 * KB], ng[:],
                                        op=ALU.mult)
                nc.vector.tensor_tensor(hC_new[:], ng[:], d[:],
                                        op=ALU.subtract)
            else:
                nc.vector.tensor_tensor(d[:], hC[:], ng[:], op=ALU.subtract)
                zd = wk.tile([P, KB], f32, tag="zd", name="zd")
                nc.vector.tensor_tensor(zd[:], rz[:, KB:2 * KB], d[:],
                                        op=ALU.mult)
                nc.vector.tensor_tensor(hC_new[:], ng[:], zd[:], op=ALU.add)
            hC = hC_new
            hB_new = hhp.tile([P, KB], bf16, tag="hB", name="hB")
            nc.vector.tensor_copy(hB_new[:], hC[:])
            hB = hB_new
            hB8_new = hhp.tile([P, KB], fp8, tag="hB8", name="hB8")
            nc.vector.tensor_copy(hB8_new[:], hC[:])
            hB8 = hB8_new

        nc.sync.dma_start(h_out, hC[:])

    nc.compile()
    return nc


def _bf(x):
    return np.ascontiguousarray(np.asarray(x, dtype=ml_dtypes.bfloat16))


def _f8(x):
    return np.ascontiguousarray(np.asarray(x, dtype=ml_dtypes.float8_e4m3))


def _f32(x):
    return np.ascontiguousarray(np.asarray(x, dtype=np.float32))


def _kblocks(W):
    """[H, N] -> [128, KB*N]: block kb = W[kb*128:(kb+1)*128, :]."""
    N = W.shape[1]
    return np.ascontiguousarray(
        W.reshape(KB, P, N).transpose(1, 0, 2).reshape(P, KB * N))


def _halfpack(W):
    """[H, H] -> [128, 2*KB*512]: half jh, block kb = W[kb-rows, jh-cols]."""
    X = W.reshape(KB, P, 2, 512)           # [kb, p, jh, 512]
    return np.ascontiguousarray(
        X.transpose(1, 2, 0, 3).reshape(P, 2 * KB * 512))


def _memT_blk(M):
    """[T, H] -> [128, KB*T]: block kb holds M.T[kb*128:(kb+1)*128, :]."""
    T = M.shape[0]
    X = np.ascontiguousarray(M.T)
    return np.ascontiguousarray(
        X.reshape(KB, P, T).transpose(1, 0, 2).reshape(P, KB * T))


def _colblk(v):
    return np.ascontiguousarray(v.reshape(KB, P).T)


def _prep_inputs(inputs):
    mem_v = _f32(inputs["memory_vid"])
    mem_t = _f32(inputs["memory_text"])
    Wav, Uav, bav, Vav = (_f32(inputs[k]) for k in ("Wav", "Uav", "bav", "Vav"))
    Wat, Uat, bat, Vat = (_f32(inputs[k]) for k in ("Wat", "Uat", "bat", "Vat"))
    Wb, bb = _f32(inputs["Wb"]), _f32(inputs["bb"])
    Whh, Wvh, Wth, bhh = (_f32(inputs[k]) for k in ("Whh", "Wvh", "Wth", "bhh"))
    W_ih, W_hh = _f32(inputs["W_ih"]), _f32(inputs["W_hh"])
    b_ih, b_hh = _f32(inputs["b_ih"]), _f32(inputs["b_hh"])

    wavT_b = _bf(_halfpack(Wav))
    watT_b = _bf(_halfpack(Wat))
    uavR_b = _f8(_kblocks(Uav))
    uatR_b = _f8(_kblocks(Uat))
    wvhR_b = _bf(_kblocks(Wvh))
    wthR_b = _bf(_kblocks(Wth))
    wihTR_b = _bf(_kblocks(np.ascontiguousarray(W_ih.T)))

    vavB_b, vatB_b = _bf(_colblk(Vav)), _bf(_colblk(Vat))
    bavB_b, batB_b = _f32(_colblk(bav)), _f32(_colblk(bat))
    bhhF_b = _f32(bhh.reshape(KB, P))
    gbF_b = _f32(np.concatenate([
        _colblk(b_ih[0:H] + b_hh[0:H]),
        _colblk(b_ih[H:2 * H] + b_hh[H:2 * H]),
        _colblk(b_ih[2 * H:3 * H]),
        _colblk(b_hh[2 * H:3 * H]),
    ], axis=1))
    bbS_b = _f32(bb.reshape(1, 2))
    ones18 = np.ones((1, KB), np.float32)
    eyeB = _bf(np.eye(32, dtype=np.float32))
    eyeF = _f32(np.eye(32, dtype=np.float32))

    in_maps = []
    for c in range(NCORES):
        svc = slice(c * TVC, (c + 1) * TVC)
        stc = slice(c * TTC, (c + 1) * TTC)
        cslice = slice(c * P, (c + 1) * P)
        mv_c, mt_c = mem_v[svc], mem_t[stc]
        memRv_b = _bf(mv_c.reshape(SVB, P, H).transpose(1, 0, 2)
                      .reshape(P, SVB * H))
        memRt_b = _bf(mt_c.reshape(STB, P, H).transpose(1, 0, 2)
                      .reshape(P, STB * H))
        whhTs = np.concatenate(
            [np.ascontiguousarray(W_hh[:, cslice].T),   # [128, 3H]
             np.ascontiguousarray(Whh[cslice, :]),      # [128, H]
             np.ascontiguousarray(Wb[cslice, :])], axis=1)
        maskB = np.zeros((P, KB), np.float32)
        maskB[:, c] = 1.0
        in_maps.append({
            "memTv": _bf(_memT_blk(mv_c)),
            "memTt": _bf(_memT_blk(mt_c)),
            "memRv": memRv_b, "memRt": memRt_b,
            "wavT": wavT_b, "watT": watT_b,
            "uavR": uavR_b, "uatR": uatR_b,
            "wvhR": wvhR_b, "wthR": wthR_b, "wihTR": wihTR_b,
            "whhTs": _bf(whhTs),
            "vavB": vavB_b, "vatB": vatB_b,
            "bavB": bavB_b, "batB": batB_b,
            "bhhF": bhhF_b, "gbF": gbF_b, "bbS": bbS_b,
            "maskB": _bf(maskB), "ones18": ones18,
            "eyeB": eyeB, "eyeF": eyeF,
        })
    return in_maps


TRACE = False
LAST_RESULT = None


def kernel(**inputs):
    global LAST_RESULT
    from concourse import bass_utils
    loop_n = int(np.asarray(inputs["loop"]))
    if loop_n not in _cache:
        _cache[loop_n] = _build(loop_n)
    nc = _cache[loop_n]
    in_maps = _prep_inputs(inputs)
    kw = {}
    if TRACE:
        import tempfile
        kw = dict(trace=True, tmpdir=tempfile.mkdtemp(prefix="bassprof_"))
    res = bass_utils.run_bass_kernel_spmd(nc, in_maps,
                                          core_ids=list(range(NCORES)), **kw)
    LAST_RESULT = res
    hC = res.results[0]["h_out"]  # [128, 8] col-block layout
    h = np.ascontiguousarray(hC.T.reshape(H))
    return h.reshape(1, H).astype(np.float32)


# revision 13
# speedup vs baseline: 1.0103x; 1.0103x over previous
"""Trainium2 Bass kernel v2 for HME-VideoQA multi-modal attention GRU.

Design vs baseline:
- ONE AllReduce per iteration: payload [cv | ct | gh-partials | hWhh-partials
  | hWb-partials | Z]. Small weights (Uav/Uat/W_ihT/Wvh/Wth) replicated per
  core so hu/gi/u GEMVs are local; h kept fully replicated on every core.
- All GEMVs row-form (vector stationary lhsT [128,1], weight streams as rhs)
  with 4-way PE column tiling (tile_position) -> 4 concurrent 512-col chains.
- bf16 streams (U matrices fp8), fp32 PSUM accumulation, fp32 gate math.
- Layout changes via fold-DMAs (contiguous segments) + PE transposes.
"""

import numpy as np
import ml_dtypes
from contextlib import ExitStack

H = 1024
P = 128
NCORES = 8
KB = H // P             # 8 H-blocks
TVC = 8192 // NCORES    # 1024 video slots/core
TTC = 2048 // NCORES    # 256 text slots/core
SVB = TVC // P          # 8 video slot blocks
STB = TTC // P          # 2 text slot blocks

# AR payload offsets (fp32). it>0: [cv|ct|gh|hwhh|wb|z]; it0: [cv|ct|z]
AB_CV, AB_CT, AB_GH, AB_HW, AB_WB, AB_Z = 0, 1024, 2048, 5120, 6144, 6146
AB_LEN = 6149
A0_Z = 2048
A0_LEN = 2051

_cache = {}


def _build(loop_n):
    import concourse.bacc as bacc
    import concourse.mybir as mybir
    import concourse.tile as tile
    import concourse.bass as bass  # noqa: F401

    nc = bacc.Bacc("TRN2", target_bir_lowering=False, debug=False,
                   num_devices=NCORES)
    f32 = mybir.dt.float32
    bf16 = mybir.dt.bfloat16
    fp8 = mybir.dt.float8e4
    AF = mybir.ActivationFunctionType
    ALU = mybir.AluOpType
    RG = [list(range(NCORES))]

    def din(name, shape, dty):
        return nc.dram_tensor(name, list(shape), dty,
                              kind="ExternalInput").ap()

    memTv_in = din("memTv", [P, KB * TVC], bf16)
    memTt_in = din("memTt", [P, KB * TTC], bf16)
    memRv_in = din("memRv", [P, SVB * H], bf16)
    memRt_in = din("memRt", [P, STB * H], bf16)
    wavT_in = din("wavT", [P, KB * H], bf16)
    watT_in = din("watT", [P, KB * H], bf16)
    uavR_in = din("uavR", [P, KB * H], fp8)
    uatR_in = din("uatR", [P, KB * H], fp8)
    wvhR_in = din("wvhR", [P, KB * H], bf16)
    wthR_in = din("wthR", [P, KB * H], bf16)
    wihTR_in = din("wihTR", [P, KB * 3 * H], bf16)
    # sharded rhs for h-partials: [W_hh.T rows | Whh rows | Wb rows]
    whhTs_in = din("whhTs", [P, 4 * H + 2], bf16)
    vavB_in = din("vavB", [P, KB], bf16)
    vatB_in = din("vatB", [P, KB], bf16)
    bavB_in = din("bavB", [P, KB], f32)
    batB_in = din("batB", [P, KB], f32)
    bhhB_in = din("bhhB", [P, KB], f32)
    gbF_in = din("gbF", [P, 4 * KB], f32)
    bbS_in = din("bbS", [1, 2], f32)
    maskB_in = din("maskB", [P, KB], bf16)
    ones1p_in = din("ones1p", [1, P], f32)
    eyeB_in = din("eyeB", [32, 32], bf16)
    eyeF_in = din("eyeF", [32, 32], f32)
    eyeF128_in = din("eyeF128", [P, P], f32)
    h_out = nc.dram_tensor("h_out", [P, KB], f32, kind="ExternalOutput").ap()

    with tile.TileContext(nc) as tc, ExitStack() as ctx:
        cst = ctx.enter_context(tc.tile_pool(name="cst", bufs=1))
        wgt = ctx.enter_context(tc.tile_pool(name="wgt", bufs=1))
        res = ctx.enter_context(tc.tile_pool(name="res", bufs=1))
        dram = ctx.enter_context(tc.tile_pool(name="dram", bufs=2,
                                              space="DRAM"))
        pbig = ctx.enter_context(tc.tile_pool(name="pbig", bufs=4,
                                              space="PSUM"))
        psm = ctx.enter_context(tc.tile_pool(name="psm", bufs=2,
                                             space="PSUM"))
        psc = ctx.enter_context(tc.tile_pool(name="psc", bufs=2,
                                             space="PSUM"))
        thp = ctx.enter_context(tc.tile_pool(name="thp", bufs=2))
        wk = ctx.enter_context(tc.tile_pool(name="wk", bufs=1))
        hhp = ctx.enter_context(tc.tile_pool(name="hh", bufs=2))
        stg = ctx.enter_context(tc.tile_pool(name="stg", bufs=3))

        # memT first on the sync ring: it gates the setup GEMM. The small
        # constants ride behind it (not needed until iter0's tanh).
        memT = wgt.tile([P, KB * TVC], bf16, tag="memT", name="memT")
        nc.sync.dma_start(memT[:], memTv_in)

        # ---- constants (scalar ring; sync ring carries the big loads) ----
        def cload(pool, ap_in, shape, dty, tag):
            t = pool.tile(list(shape), dty, tag=tag, name=tag)
            nc.scalar.dma_start(t[:], ap_in)
            return t

        vavB = cload(cst, vavB_in, [P, KB], bf16, "vavB")
        vatB = cload(cst, vatB_in, [P, KB], bf16, "vatB")
        bavB = cload(cst, bavB_in, [P, KB], f32, "bavB")
        batB = cload(cst, batB_in, [P, KB], f32, "batB")
        bhhB = cload(cst, bhhB_in, [P, KB], f32, "bhhB")
        gbF = cload(cst, gbF_in, [P, 4 * KB], f32, "gbF")
        bbS = cload(cst, bbS_in, [1, 2], f32, "bbS")
        maskB = cload(cst, maskB_in, [P, KB], bf16, "maskB")
        ones1p = cload(cst, ones1p_in, [1, P], f32, "ones1p")
        eyeB = cload(cst, eyeB_in, [32, 32], bf16, "eyeB")
        eyeF = cload(cst, eyeF_in, [32, 32], f32, "eyeF")
        eyeF128 = cload(cst, eyeF128_in, [P, P], f32, "eyeF128")

        zeros8F = cst.tile([KB, P], f32, tag="zeros8F", name="zeros8F")
        nc.vector.memset(zeros8F[:], 0.0)
        wrm = cst.tile([P, P], bf16, tag="wrm", name="wrm")
        nc.vector.memset(wrm[:], 0.0)
        zeros12 = cst.tile([1, 2], f32, tag="zeros12", name="zeros12")
        nc.vector.memset(zeros12[:], 0.0)
        zeros24 = cst.tile([P, 3 * KB], f32, tag="zeros24", name="zeros24")
        nc.vector.memset(zeros24[:], 0.0)

        # ---- big DMAs (sync ring, in need order) ----
        memTtS = wgt.tile([P, KB * TTC], bf16, tag="memTt2", name="memTtS")
        nc.sync.dma_start(memTtS[:], memTt_in)

        mvw = res.tile([P, KB * TVC], bf16, tag="mvw", name="mvw")
        mtw = res.tile([P, KB * TTC], bf16, tag="mtw", name="mtw")

        def mm(out, lhsT, rhs, tp, start, stop):
            nc.tensor.matmul(out, lhsT, rhs, start=start, stop=stop,
                             tile_position=tp, skip_group_check=True)

        def flush(bank):
            """DVE-copy a PSUM bank to SBUF (DMA cannot read PSUM)."""
            s = stg.tile([P, 512], f32, tag="stg", name="s")
            nc.vector.tensor_copy(s[:], bank[:])
            return s

        # ---- setup GEMM (video): mvw[jb-block] = (mem@Wav).T layout ----
        # w half jh: blocks kb hold Wav[kb-rows, jh*512:(jh+1)*512]
        for jh in range(2):
            wv = wgt.tile([P, KB * 512], bf16, tag="wtag", name="wv",
                          bufs=2)
            nc.sync.dma_start(wv[:], wavT_in[:, jh * 4096:(jh + 1) * 4096])
            for jj in range(4):
                jb = jh * 4 + jj
                for pc in range(2):
                    ps = pbig.tile([P, 512], f32, tag="big", name="ps")
                    for kb in range(KB):
                        nc.tensor.matmul(
                            ps[:],
                            wv[:, kb * 512 + jj * P: kb * 512 + (jj + 1) * P],
                            memT[:, kb * TVC + pc * 512:
                                 kb * TVC + (pc + 1) * 512],
                            start=(kb == 0), stop=(kb == KB - 1))
                    nc.vector.tensor_copy(
                        mvw[:, jb * TVC + pc * 512: jb * TVC + (pc + 1) * 512],
                        ps[:])

        # text GEMM
        for jh in range(2):
            wt = wgt.tile([P, KB * 512], bf16, tag="wtag", name="wt",
                          bufs=2)
            nc.sync.dma_start(wt[:], watT_in[:, jh * 4096:(jh + 1) * 4096])
            for jj in range(4):
                jb = jh * 4 + jj
                ps = pbig.tile([P, TTC], f32, tag="big", name="ps")
                for kb in range(KB):
                    nc.tensor.matmul(
                        ps[:],
                        wt[:, kb * 512 + jj * P: kb * 512 + (jj + 1) * P],
                        memTtS[:, kb * TTC: (kb + 1) * TTC],
                        start=(kb == 0), stop=(kb == KB - 1))
                nc.vector.tensor_copy(mtw[:, jb * TTC:(jb + 1) * TTC], ps[:])

        # remaining weights (needed from iter0 context onward)
        memRv = wgt.tile([P, SVB * H], bf16, tag="memRv", name="memRv")
        nc.scalar.dma_start(memRv[:], memRv_in)
        memRt = wgt.tile([P, STB * H], bf16, tag="memRt", name="memRt")
        nc.scalar.dma_start(memRt[:], memRt_in)
        wvhR = wgt.tile([P, KB * H], bf16, tag="wvhR", name="wvhR")
        nc.scalar.dma_start(wvhR[:], wvhR_in)
        wthR = wgt.tile([P, KB * H], bf16, tag="wthR", name="wthR")
        nc.scalar.dma_start(wthR[:], wthR_in)
        wihTR = wgt.tile([P, KB * 3 * H], bf16, tag="wihTR", name="wihTR")
        nc.sync.dma_start(wihTR[:], wihTR_in)
        uavR = wgt.tile([P, KB * H], fp8, tag="uavR", name="uavR")
        nc.sync.dma_start(uavR[:], uavR_in)
        uatR = wgt.tile([P, KB * H], fp8, tag="uatR", name="uatR")
        nc.sync.dma_start(uatR[:], uatR_in)
        whhTs = wgt.tile([P, 4 * H + 2], bf16, tag="whhTs", name="whhTs")
        nc.sync.dma_start(whhTs[:], whhTs_in)

        # ---- recurrence ----
        hC = None     # [P, KB] f32, full h, col-block layout
        hB = None     # bf16 copy
        hB8 = None    # fp8 copy (lhsT for fp8 U GEMVs)

        for it in range(loop_n):
            first = (it == 0)

            if not first:
                # --- phase 1a: local hu GEMVs (fp8 U, replicated) ---
                g1 = pbig.tile([P, 512], f32, tag="big", name="g1")
                rhs_map = [(uavR, 0), (uavR, 512), (uatR, 0), (uatR, 512)]
                for kb in range(KB):
                    hcol = hB8[:, kb:kb + 1]
                    for j, (w, off) in enumerate(rhs_map):
                        mm(g1[32 * j:32 * j + 1, :], hcol,
                           w[:, kb * H + off: kb * H + off + 512],
                           (0, 32 * j), kb == 0, kb == KB - 1)

                # --- phase 1b: sharded partials (gh, hWhh, hWb) ---
                hsel = wk.tile([P, 1], f32, tag="hsel", name="hsel")
                msk = wk.tile([P, KB], f32, tag="msk", name="msk")
                nc.vector.tensor_tensor(msk[:], hB[:], maskB[:], op=ALU.mult)
                nc.vector.tensor_reduce(hsel[:], msk[:],
                                        axis=mybir.AxisListType.XYZW,
                                        op=ALU.add)
                hselB = wk.tile([P, 1], bf16, tag="hselB", name="hselB")
                nc.vector.tensor_copy(hselB[:], hsel[:])

                g3 = pbig.tile([P, 512], f32, tag="big", name="g3")
                g4 = pbig.tile([P, 512], f32, tag="big", name="g4")
                for j in range(4):
                    mm(g3[32 * j:32 * j + 1, :], hselB[:],
                       whhTs[:, j * 512:(j + 1) * 512],
                       (0, 32 * j), True, True)
                for j in range(4):
                    mm(g4[32 * j:32 * j + 1, :], hselB[:],
                       whhTs[:, (4 + j) * 512:(5 + j) * 512],
                       (0, 32 * j), True, True)
                pwb = psm.tile([P, 32], f32, tag="smF", name="pwb")
                mm(pwb[0:1, 0:2], hselB[:], whhTs[:, 4 * H:4 * H + 2],
                   (0, 0), True, True)

                # fold hu FIRST (gates the tanh phase; scalar ring)
                sg1 = flush(g1)
                huF = wk.tile([2 * KB, P], f32, tag="huF", name="huF")
                nc.scalar.dma_start(huF[:], sg1[0:128:32, :])

                # stage partials into AR input (gpsimd ring, needed later)
                sg3 = flush(g3)
                sg4 = flush(g4)
                spwb = wk.tile([1, 2], f32, tag="spwb", name="spwb")
                nc.vector.tensor_copy(spwb[:], pwb[0:1, 0:2])
                arin = dram.tile([1, AB_LEN], f32, tag="arin", name="arin")
                nc.scalar.dma_start(arin[0, AB_GH:AB_GH + 2048],
                                    sg3[0:128:32, :])
                nc.scalar.dma_start(arin[0, AB_GH + 2048:AB_GH + 4096],
                                    sg4[0:128:32, :])
                nc.scalar.dma_start(arin[0, AB_WB:AB_WB + 2], spwb[:])

                pt = psm.tile([P, 32], f32, tag="smF", name="pt")
                nc.tensor.transpose(pt[:, 0:2 * KB], huF[:],
                                    eyeF[0:2 * KB, 0:2 * KB])
                biasV = wk.tile([P, KB], f32, tag="biasV", name="biasV")
                biasT = wk.tile([P, KB], f32, tag="biasT", name="biasT")
                nc.vector.tensor_tensor(biasV[:], pt[:, 0:KB], bavB[:],
                                        op=ALU.add)
                nc.vector.tensor_tensor(biasT[:], pt[:, KB:2 * KB], batB[:],
                                        op=ALU.add)
            else:
                biasV, biasT = bavB, batB
                arin = dram.tile([1, A0_LEN], f32, tag="arin0", name="arin0")

            # --- tanh + scores ---
            sc = pbig.tile([P, 512], f32, tag="big", name="sc")
            for kb in range(KB):
                th = thp.tile([P, TVC], bf16, tag="thv", name="th")
                nc.scalar.activation(th[:], mvw[:, kb * TVC:(kb + 1) * TVC],
                                     AF.Tanh, bias=biasV[:, kb:kb + 1])
                mm(sc[0:1, :], vavB[:, kb:kb + 1], th[:, 0:512],
                   (0, 0), kb == 0, kb == KB - 1)
                mm(sc[32:33, :], vavB[:, kb:kb + 1], th[:, 512:1024],
                   (0, 32), kb == 0, kb == KB - 1)
            for kb in range(KB):
                tht = thp.tile([P, TTC], bf16, tag="tht", name="tht")
                nc.scalar.activation(tht[:], mtw[:, kb * TTC:(kb + 1) * TTC],
                                     AF.Tanh, bias=biasT[:, kb:kb + 1])
                mm(sc[64:65, 0:TTC], vatB[:, kb:kb + 1], tht[:],
                   (0, 64), kb == 0, kb == KB - 1)

            # --- exp + Z ---
            evS = wk.tile([33, 512], bf16, tag="evS", name="evS")
            accS = wk.tile([33, 1], f32, tag="accS", name="accS")
            nc.scalar.activation(evS[:], sc[0:33, :], AF.Exp,
                                 accum_out=accS[:])
            etS = wk.tile([1, TTC], bf16, tag="etS", name="etS")
            accT = wk.tile([1, 1], f32, tag="accT", name="accT")
            nc.scalar.activation(etS[:], sc[64:65, 0:TTC], AF.Exp,
                                 accum_out=accT[:])
            zoff = A0_Z if first else AB_Z
            nc.scalar.dma_start(arin[0, zoff:zoff + 1], accS[0:1, :])
            nc.scalar.dma_start(arin[0, zoff + 1:zoff + 2], accS[32:33, :])
            nc.scalar.dma_start(arin[0, zoff + 2:zoff + 3], accT[0:1, :])

            # fold ev -> [10,128], transpose -> evT [128,10]
            evF = wk.tile([SVB + STB, P], bf16, tag="evF", name="evF")
            nc.scalar.dma_start(evF[0:4, :], evS[0:1, :])
            nc.scalar.dma_start(evF[4:8, :], evS[32:33, :])
            nc.scalar.dma_start(evF[8:10, :], etS[:])
            ptev = psm.tile([P, 32], bf16, tag="smF", name="ptev")
            nc.tensor.transpose(ptev[:, 0:SVB + STB], evF[:],
                                eyeB[0:SVB + STB, 0:SVB + STB])
            evT = wk.tile([P, SVB + STB], bf16, tag="evT", name="evT")
            nc.vector.tensor_copy(evT[:], ptev[:, 0:SVB + STB])

            # --- context ---
            cx = pbig.tile([P, 512], f32, tag="big", name="cx")
            for sb in range(SVB):
                mm(cx[0:1, :], evT[:, sb:sb + 1],
                   memRv[:, sb * H: sb * H + 512],
                   (0, 0), sb == 0, sb == SVB - 1)
                mm(cx[32:33, :], evT[:, sb:sb + 1],
                   memRv[:, sb * H + 512: (sb + 1) * H],
                   (0, 32), sb == 0, sb == SVB - 1)
            for sb in range(STB):
                mm(cx[64:65, :], evT[:, SVB + sb:SVB + sb + 1],
                   memRt[:, sb * H: sb * H + 512],
                   (0, 64), sb == 0, sb == STB - 1)
                mm(cx[96:97, :], evT[:, SVB + sb:SVB + sb + 1],
                   memRt[:, sb * H + 512: (sb + 1) * H],
                   (0, 96), sb == 0, sb == STB - 1)
            scx = flush(cx)
            nc.scalar.dma_start(arin[0, 0:2048], scx[0:128:32, :])

            # --- AllReduce ---
            if first:
                arout = dram.tile([1, A0_LEN], f32, tag="arout0",
                                  name="arout0", addr_space="Shared")
            else:
                arout = dram.tile([1, AB_LEN], f32, tag="arout",
                                  name="arout", addr_space="Shared")
            nc.gpsimd.collective_compute(
                "AllReduce", ALU.add, replica_groups=RG,
                ins=[arin.opt()], outs=[arout.opt()])

            # --- unstage ---
            cvctF = wk.tile([2 * KB, P], f32, tag="cvctF", name="cvctF")
            nc.scalar.dma_start(cvctF[:], arout[0, AB_CV:AB_CV + 2 * H])
            zS = wk.tile([1, 3], f32, tag="zS", name="zS")
            nc.scalar.dma_start(zS[:], arout[0, zoff:zoff + 3])
            if not first:
                ghF = wk.tile([3 * KB, P], f32, tag="ghF", name="ghF")
                nc.sync.dma_start(ghF[:], arout[0, AB_GH:AB_GH + 3 * H])
                hWhhF = wk.tile([KB, P], f32, tag="hWhhF", name="hWhhF")
                nc.sync.dma_start(hWhhF[:], arout[0, AB_HW:AB_HW + H])
                hwbS = wk.tile([1, 2], f32, tag="hwbS", name="hwbS")
                nc.scalar.dma_start(hwbS[:], arout[0, AB_WB:AB_WB + 2])
            else:
                hwbS = zeros12
                hWhhF = zeros8F

            # --- beta / Z scalars ---
            bsum = wk.tile([1, 2], f32, tag="bsum", name="bsum")
            nc.vector.tensor_tensor(bsum[:], hwbS[:], bbS[:], op=ALU.add)
            eb = wk.tile([1, 2], f32, tag="eb", name="eb")
            ebs = wk.tile([1, 1], f32, tag="ebs", name="ebs")
            nc.scalar.activation(eb[:], bsum[:], AF.Exp, accum_out=ebs[:])
            erec = wk.tile([1, 1], f32, tag="erec", name="erec")
            nc.vector.reciprocal(erec[:], ebs[:])
            beta = wk.tile([1, 2], f32, tag="beta", name="beta")
            nc.vector.tensor_scalar_mul(beta[:], eb[:], erec[:])
            zz = wk.tile([1, 2], f32, tag="zz", name="zz")
            nc.vector.tensor_tensor(zz[:, 0:1], zS[:, 0:1], zS[:, 1:2],
                                    op=ALU.add)
            nc.vector.tensor_copy(zz[:, 1:2], zS[:, 2:3])
            zrec = wk.tile([1, 2], f32, tag="zrec", name="zrec")
            nc.vector.reciprocal(zrec[:], zz[:])
            rr = wk.tile([1, 2], f32, tag="rr", name="rr")
            nc.vector.tensor_tensor(rr[:], beta[:], zrec[:], op=ALU.mult)
            prr = psm.tile([P, 32], f32, tag="smF", name="prr")
            nc.tensor.matmul(prr[:, 0:2], ones1p[:], rr[:],
                             start=True, stop=True)
            rrB = wk.tile([P, 2], f32, tag="rrB", name="rrB")
            nc.vector.tensor_copy(rrB[:], prr[:, 0:2])

            # --- u-GEMV: u = cv @ Wvh, ut = ct @ Wth (replicated) ---
            ptcv = psm.tile([P, 32], f32, tag="smF", name="ptcv")
            nc.tensor.transpose(ptcv[:, 0:2 * KB], cvctF[:],
                                eyeF[0:2 * KB, 0:2 * KB])
            cvctB = wk.tile([P, 2 * KB], bf16, tag="cvctB", name="cvctB")
            nc.vector.tensor_copy(cvctB[:], ptcv[:, 0:2 * KB])
            ub = pbig.tile([P, 512], f32, tag="big", name="ub")
            for kb in range(KB):
                mm(ub[0:1, :], cvctB[:, kb:kb + 1],
                   wvhR[:, kb * H: kb * H + 512],
                   (0, 0), kb == 0, kb == KB - 1)
                mm(ub[32:33, :], cvctB[:, kb:kb + 1],
                   wvhR[:, kb * H + 512: (kb + 1) * H],
                   (0, 32), kb == 0, kb == KB - 1)
                mm(ub[64:65, :], cvctB[:, KB + kb:KB + kb + 1],
                   wthR[:, kb * H: kb * H + 512],
                   (0, 64), kb == 0, kb == KB - 1)
                mm(ub[96:97, :], cvctB[:, KB + kb:KB + kb + 1],
                   wthR[:, kb * H + 512: (kb + 1) * H],
                   (0, 96), kb == 0, kb == KB - 1)

            # --- mm_o in column layout [128, 8] via PE chunk-transposes ---
            sub = flush(ub)
            uvC = wk.tile([P, KB], f32, tag="uvC", name="uvC")
            utC = wk.tile([P, KB], f32, tag="utC", name="utC")
            for j in range(4):
                tpU = psc.tile([P, P], f32, tag="psc", name="tpU")
                nc.tensor.transpose(tpU[:], sub[:, j * P:(j + 1) * P],
                                    eyeF128[:])
                nc.vector.tensor_copy(uvC[:, j:j + 5:4], tpU[:, 0:33:32])
                nc.vector.tensor_copy(utC[:, j:j + 5:4], tpU[:, 64:97:32])
            if first:
                hwbC = bhhB
            else:
                tpW = psm.tile([P, 32], f32, tag="smF", name="tpW")
                nc.tensor.transpose(tpW[:, 0:KB], hWhhF[:],
                                    eyeF[0:KB, 0:KB])
                hwbC = wk.tile([P, KB], f32, tag="hwbC", name="hwbC")
                nc.vector.tensor_tensor(hwbC[:], tpW[:, 0:KB], bhhB[:],
                                        op=ALU.add)
            t1 = wk.tile([P, KB], f32, tag="t1", name="t1")
            nc.vector.scalar_tensor_tensor(t1[:], uvC[:], rrB[:, 0:1],
                                           hwbC[:], op0=ALU.mult,
                                           op1=ALU.add)
            t2 = wk.tile([P, KB], f32, tag="t2", name="t2")
            nc.vector.scalar_tensor_tensor(t2[:], utC[:],
                                           rrB[:, 1:2], t1[:],
                                           op0=ALU.mult, op1=ALU.add)
            moB = wk.tile([P, KB], bf16, tag="moB", name="moB")
            nc.scalar.activation(moB[:], t2[:], AF.Tanh)

            # --- gi GEMV: gi = mo @ W_ih.T (full 3H, local) ---
            giE = pbig.tile([P, 512], f32, tag="big", name="giE")
            giF_ = pbig.tile([P, 512], f32, tag="big", name="giF_")
            for kb in range(KB):
                mo_col = moB[:, kb:kb + 1]
                for pc in range(4):
                    mm(giE[32 * pc:32 * pc + 1, :], mo_col,
                       wihTR[:, kb * 3 * H + pc * 512:
                             kb * 3 * H + (pc + 1) * 512],
                       (0, 32 * pc), kb == 0, kb == KB - 1)
                for pc in range(2):
                    mm(giF_[32 * pc:32 * pc + 1, :], mo_col,
                       wihTR[:, kb * 3 * H + (4 + pc) * 512:
                             kb * 3 * H + (5 + pc) * 512],
                       (0, 32 * pc), kb == 0, kb == KB - 1)

            # gi -> columns [128, 24] via PE chunk-transposes
            sgiE = flush(giE)
            sgiF = flush(giF_)
            giC = wk.tile([P, 3 * KB], f32, tag="giC", name="giC")
            for j in range(4):
                tpE = psc.tile([P, P], f32, tag="psc", name="tpE")
                nc.tensor.transpose(tpE[:], sgiE[:, j * P:(j + 1) * P],
                                    eyeF128[:])
                nc.vector.tensor_copy(giC[:, j:16 + j:4], tpE[:, 0:128:32])
                tpF2 = psc.tile([P, P], f32, tag="psc", name="tpF2")
                nc.tensor.transpose(tpF2[:], sgiF[:, j * P:(j + 1) * P],
                                    eyeF128[:])
                nc.vector.tensor_copy(giC[:, 16 + j:24:4], tpF2[:, 0:64:32])

            if not first:
                ptgh = psm.tile([P, 32], f32, tag="smF", name="ptgh")
                nc.tensor.transpose(ptgh[:, 0:3 * KB], ghF[:],
                                    eyeF[0:3 * KB, 0:3 * KB])
                ghC = wk.tile([P, 3 * KB], f32, tag="ghC", name="ghC")
                nc.vector.tensor_copy(ghC[:], ptgh[:, 0:3 * KB])
            else:
                ghC = zeros24

            # --- gates (columns; r cols 0-7, z 8-15, n 16-23) ---
            pre = wk.tile([P, 2 * KB], f32, tag="pre", name="pre")
            nc.vector.tensor_tensor(pre[:], giC[:, 0:2 * KB],
                                    ghC[:, 0:2 * KB], op=ALU.add)
            nc.vector.tensor_tensor(pre[:], pre[:], gbF[:, 0:2 * KB],
                                    op=ALU.add)
            erz = wk.tile([P, 2 * KB], f32, tag="erz", name="erz")
            nc.scalar.activation(erz[:], pre[:], AF.Exp, scale=-1.0)
            nc.vector.tensor_scalar_add(erz[:], erz[:], 1.0)
            rz = wk.tile([P, 2 * KB], f32, tag="rz", name="rz")
            nc.vector.reciprocal(rz[:], erz[:])
            hn = wk.tile([P, KB], f32, tag="hn", name="hn")
            nc.vector.tensor_tensor(hn[:], ghC[:, 2 * KB:3 * KB],
                                    gbF[:, 3 * KB:4 * KB], op=ALU.add)
            m1 = wk.tile([P, KB], f32, tag="m1", name="m1")
            nc.vector.tensor_tensor(m1[:], rz[:, 0:KB], hn[:], op=ALU.mult)
            tn = wk.tile([P, KB], f32, tag="tn", name="tn")
            nc.vector.tensor_tensor(tn[:], giC[:, 2 * KB:3 * KB],
                                    gbF[:, 2 * KB:3 * KB], op=ALU.add)
            nc.vector.tensor_tensor(tn[:], tn[:], m1[:], op=ALU.add)
            ng = wk.tile([P, KB], f32, tag="ng", name="ng")
            nc.scalar.activation(ng[:], tn[:], AF.Tanh)
            hC_new = hhp.tile([P, KB], f32, tag="hC", name="hC")
            d = wk.tile([P, KB], f32, tag="d", name="d")
            if first:
                nc.vector.tensor_tensor(d[:], rz[:, KB:2 * KB], ng[:],
                                        op=ALU.mult)
                nc.vector.tensor_tensor(hC_new[:], ng[:], d[:],
                                        op=ALU.subtract)
            else:
                nc.vector.tensor_tensor(d[:], hC[:], ng[:], op=ALU.subtract)
                zd = wk.tile([P, KB], f32, tag="zd", name="zd")
                nc.vector.tensor_tensor(zd[:], rz[:, KB:2 * KB], d[:],
                                        op=ALU.mult)
                nc.vector.tensor_tensor(hC_new[:], ng[:], zd[:], op=ALU.add)
            hC = hC_new
            hB_new = hhp.tile([P, KB], bf16, tag="hB", name="hB")
            nc.vector.tensor_copy(hB_new[:], hC[:])
            hB = hB_new
            hB8_new = hhp.tile([P, KB], fp8, tag="hB8", name="hB8")
            nc.vector.tensor_copy(hB8_new[:], hC[:])
            hB8 = hB8_new

        nc.sync.dma_start(h_out, hC[:])

    nc.compile()
    return nc


def _bf(x):
    return np.ascontiguousarray(np.asarray(x, dtype=ml_dtypes.bfloat16))


def _f8(x):
    return np.ascontiguousarray(np.asarray(x, dtype=ml_dtypes.float8_e4m3))


def _f32(x):
    return np.ascontiguousarray(np.asarray(x, dtype=np.float32))


def _kblocks(W):
    """[H, N] -> [128, KB*N]: block kb = W[kb*128:(kb+1)*128, :]."""
    N = W.shape[1]
    return np.ascontiguousarray(
        W.reshape(KB, P, N).transpose(1, 0, 2).reshape(P, KB * N))


def _halfpack(W):
    """[H, H] -> [128, 2*KB*512]: half jh, block kb = W[kb-rows, jh-cols]."""
    X = W.reshape(KB, P, 2, 512)           # [kb, p, jh, 512]
    return np.ascontiguousarray(
        X.transpose(1, 2, 0, 3).reshape(P, 2 * KB * 512))


def _memT_blk(M):
    """[T, H] -> [128, KB*T]: block kb holds M.T[kb*128:(kb+1)*128, :]."""
    T = M.shape[0]
    X = np.ascontiguousarray(M.T)
    return np.ascontiguousarray(
        X.reshape(KB, P, T).transpose(1, 0, 2).reshape(P, KB * T))


def _colblk(v):
    return np.ascontiguousarray(v.reshape(KB, P).T)


def _prep_inputs(inputs):
    mem_v = _f32(inputs["memory_vid"])
    mem_t = _f32(inputs["memory_text"])
    Wav, Uav, bav, Vav = (_f32(inputs[k]) for k in ("Wav", "Uav", "bav", "Vav"))
    Wat, Uat, bat, Vat = (_f32(inputs[k]) for k in ("Wat", "Uat", "bat", "Vat"))
    Wb, bb = _f32(inputs["Wb"]), _f32(inputs["bb"])
    Whh, Wvh, Wth, bhh = (_f32(inputs[k]) for k in ("Whh", "Wvh", "Wth", "bhh"))
    W_ih, W_hh = _f32(inputs["W_ih"]), _f32(inputs["W_hh"])
    b_ih, b_hh = _f32(inputs["b_ih"]), _f32(inputs["b_hh"])

    wavT_b = _bf(_halfpack(Wav))
    watT_b = _bf(_halfpack(Wat))
    uavR_b = _f8(_kblocks(Uav))
    uatR_b = _f8(_kblocks(Uat))
    wvhR_b = _bf(_kblocks(Wvh))
    wthR_b = _bf(_kblocks(Wth))
    wihTR_b = _bf(_kblocks(np.ascontiguousarray(W_ih.T)))

    vavB_b, vatB_b = _bf(_colblk(Vav)), _bf(_colblk(Vat))
    bavB_b, batB_b = _f32(_colblk(bav)), _f32(_colblk(bat))
    bhhB_b = _f32(_colblk(bhh))
    gbF_b = _f32(np.concatenate([
        _colblk(b_ih[0:H] + b_hh[0:H]),
        _colblk(b_ih[H:2 * H] + b_hh[H:2 * H]),
        _colblk(b_ih[2 * H:3 * H]),
        _colblk(b_hh[2 * H:3 * H]),
    ], axis=1))
    bbS_b = _f32(bb.reshape(1, 2))
    ones1p = np.ones((1, P), np.float32)
    eyeB = _bf(np.eye(32, dtype=np.float32))
    eyeF = _f32(np.eye(32, dtype=np.float32))
    eyeF128 = _f32(np.eye(P, dtype=np.float32))

    in_maps = []
    for c in range(NCORES):
        svc = slice(c * TVC, (c + 1) * TVC)
        stc = slice(c * TTC, (c + 1) * TTC)
        cslice = slice(c * P, (c + 1) * P)
        mv_c, mt_c = mem_v[svc], mem_t[stc]
        memRv_b = _bf(mv_c.reshape(SVB, P, H).transpose(1, 0, 2)
                      .reshape(P, SVB * H))
        memRt_b = _bf(mt_c.reshape(STB, P, H).transpose(1, 0, 2)
                      .reshape(P, STB * H))
        whhTs = np.concatenate(
            [np.ascontiguousarray(W_hh[:, cslice].T),   # [128, 3H]
             np.ascontiguousarray(Whh[cslice, :]),      # [128, H]
             np.ascontiguousarray(Wb[cslice, :])], axis=1)
        maskB = np.zeros((P, KB), np.float32)
        maskB[:, c] = 1.0
        in_maps.append({
            "memTv": _bf(_memT_blk(mv_c)),
            "memTt": _bf(_memT_blk(mt_c)),
            "memRv": memRv_b, "memRt": memRt_b,
            "wavT": wavT_b, "watT": watT_b,
            "uavR": uavR_b, "uatR": uatR_b,
            "wvhR": wvhR_b, "wthR": wthR_b, "wihTR": wihTR_b,
            "whhTs": _bf(whhTs),
            "vavB": vavB_b, "vatB": vatB_b,
            "bavB": bavB_b, "batB": batB_b,
            "bhhB": bhhB_b, "gbF": gbF_b, "bbS": bbS_b,
            "maskB": _bf(maskB), "ones1p": ones1p,
            "eyeB": eyeB, "eyeF": eyeF, "eyeF128": eyeF128,
        })
    return in_maps


TRACE = False
LAST_RESULT = None


def kernel(**inputs):
    global LAST_RESULT
    from concourse import bass_utils
    loop_n = int(np.asarray(inputs["loop"]))
    if loop_n not in _cache:
        _cache[loop_n] = _build(loop_n)
    nc = _cache[loop_n]
    in_maps = _prep_inputs(inputs)
    kw = {}
    if TRACE:
        import tempfile
        kw = dict(trace=True, tmpdir=tempfile.mkdtemp(prefix="bassprof_"))
    res = bass_utils.run_bass_kernel_spmd(nc, in_maps,
                                          core_ids=list(range(NCORES)), **kw)
    LAST_RESULT = res
    hC = res.results[0]["h_out"]  # [128, 8] col-block layout
    h = np.ascontiguousarray(hC.T.reshape(H))
    return h.reshape(1, H).astype(np.float32)
